# revision 11
# baseline (speedup 1.0000x reference)
"""Trainium2 Bass kernel for a dense transformer attention block (v3, bf16).

Reference computation (per batch b, tokens n=2048, d=1024, 16 heads x 64):
    xn  = LayerNorm(x) * gamma + beta
    qkv = xn @ W_qkv^T ;  q,k,v per head
    att = softmax(q k^T / sqrt(hd)) v
    out = concat_heads(att) @ W_out^T

Sharding over 8 cores: data-parallel over the 4 batches x tensor-parallel over
2 head-groups of 8 heads.  Core c handles batch c//2, heads (c%2)*8 ..+8.
Each core produces a partial out^T (its heads' contribution); the host sums
the two partials per batch and transposes back.

v3 design notes:
  - bf16 data path, fp32 PSUM accumulation and LN-stats chain (a bf16 rstd is
    a systematic per-token scale error that q.k doubles and exp amplifies).
  - NO explicit xhat pass: with gamma absorbed into W and the score scale
    folded, and wsum_f = sum_d W[f,d], the QKV projections run on RAW x^T and
    LayerNorm becomes a rank-1 correction applied during the PSUM drain:
        q[f,t] = rs[t]*(q_raw[f,t] - mu[t]*wsum_f) + c_f
    In feature-major layout mu/rs are free-axis tensors (replicated across
    partitions by the ones-matmul) and wsum/c are per-partition scalars, so
    the drain is scalar_tensor_tensor + mult + bias-add.  In token-major
    layout (V) it is two scalar_tensor_tensor ops with mu/rs as per-partition
    columns, obtained by a transposing DRAM bounce of the stats rows.
  - attention per pair: S^T with the two heads of a pair on PE row-groups
    0-63/64-127 (concurrent on HW), exp on ACT fused with the PSUM drain,
    PV with a ones-column on V giving O^T + softmax denominator in one group.
  - O stays SBUF-resident in bf16; final projection reads it directly.
  - the QK projection+drain of pair p+1 is software-pipelined INTO pair p's
    attention k-loop (engines execute their streams in order, so overlap
    must be arranged at emission time).  PSUM: S 2x2, O-accum 2, QK 2 banks.
"""

import numpy as np

import concourse.bass as bass
import concourse.mybir as mybir
import concourse.tile as tile
from concourse.alu_op_type import AluOpType

P = 128
D = 1024            # model dim
NTOK = 2048         # tokens per batch
HD = 64             # head dim
NH = 16             # total heads
NH_CORE = 8         # heads per core
INNER_C = NH_CORE * HD   # 512 inner dims per core
DCH = D // P        # 8 d-chunks of 128
KT = NTOK // P      # 16 token tiles of 128 (attention k)
NQC = NTOK // 512   # 4 q-chunks of 512
LN_EPS = 1e-5

f32 = mybir.dt.float32
bf16 = mybir.dt.bfloat16
AF = mybir.ActivationFunctionType
MUL = AluOpType.mult
ADD = AluOpType.add
BYP = AluOpType.bypass

_WCTR = [0]


def _legalize_waits(nc, max_waits=1):
    """Walrus wait-slot limits are tiny (fp32 matmul: 1). Hoist excess sync
    waits onto preceding same-engine NoOps — engines execute their stream in
    order, so this is semantics-preserving."""
    import bass_rust as _br
    for fn in nc.m.functions:
        for blk in fn.blocks:
            out = []
            for inst in blk.instructions:
                si = getattr(inst, "sync_info", None)
                if si is not None and len(si.on_wait) > max_waits:
                    waits = list(si.on_wait)
                    keep, extra = waits[:max_waits], waits[max_waits:]
                    eng = inst.engine
                    for w in extra:
                        _WCTR[0] += 1
                        nop = mybir.InstNoOp(name=f"WNOP-{_WCTR[0]}",
                                             ins=[], outs=[])
                        nop.engine = eng
                        nop.sync_info = _br.SyncInfo(on_wait=[w], on_update=[])
                        out.append(nop)
                    inst.sync_info = _br.SyncInfo(on_wait=keep,
                                                  on_update=list(si.on_update))
                out.append(inst)
            blk.instructions[:] = out


def build_nc(loop_n=None):
    nc = bass.Bass()

    xT = nc.dram_tensor("xT", [D, NTOK], bf16, kind="ExternalInput")
    # [d, 1024]: cols 0:512 = q feats (8 heads x 64), cols 512:1024 = k feats
    wqkT = nc.dram_tensor("wqkT", [D, 2 * INNER_C], bf16, kind="ExternalInput")
    wvT = nc.dram_tensor("wvT", [D, INNER_C], bf16, kind="ExternalInput")
    woT = nc.dram_tensor("woT", [INNER_C, D], bf16, kind="ExternalInput")
    onesc = nc.dram_tensor("onesc", [P, P], bf16, kind="ExternalInput")
    # per-feature bias (W @ beta): col j<4 -> q pair j, col j>=4 -> k pair j-4
    cqk = nc.dram_tensor("cqk", [P, 8], f32, kind="ExternalInput")
    # negated row-sums of the folded W (rank-1 LN mean correction)
    nws = nc.dram_tensor("nws", [P, 8], f32, kind="ExternalInput")
    cv = nc.dram_tensor("cv", [1, INNER_C], f32, kind="ExternalInput")
    nwsv = nc.dram_tensor("nwsv", [1, INNER_C], f32, kind="ExternalInput")
    outT = nc.dram_tensor("outT", [D, NTOK], f32, kind="ExternalOutput")

    with tile.TileContext(nc) as tc:
        if loop_n:
            with tc.For_i(0, loop_n, 1):
                _emit(nc, tc, xT, wqkT, wvT, woT, cqk, nws, cv, nwsv, onesc, outT)
        else:
            _emit(nc, tc, xT, wqkT, wvT, woT, cqk, nws, cv, nwsv, onesc, outT)
    _legalize_waits(nc)
    return nc


def _emit(nc, tc, xT, wqkT, wvT, woT, cqk, nws, cv, nwsv, onesc, outT):
    from contextlib import ExitStack

    es = ExitStack()
    with es:
        es.enter_context(nc.allow_low_precision(
            reason="bf16 data path; verified against the fp32 reference"))
        const = es.enter_context(tc.tile_pool(name="const", bufs=1))
        ones_sb = const.tile([P, P], bf16)
        nc.sync.dma_start(ones_sb[:], onesc[:])
        cqk_sb = const.tile([P, 8], f32)
        nc.sync.dma_start(cqk_sb[:], cqk[:])
        nws_sb = const.tile([P, 8], f32)
        nc.sync.dma_start(nws_sb[:], nws[:])
        cv_sb = const.tile([P, INNER_C], f32)
        nc.sync.dma_start(
            cv_sb[:],
            cv[0:1, :].partition_broadcast(P).rearrange("p o f -> p (o f)"))
        nwsv_sb = const.tile([P, INNER_C], f32)
        nc.sync.dma_start(
            nwsv_sb[:],
            nwsv[0:1, :].partition_broadcast(P).rearrange("p o f -> p (o f)"))
        eps_sb = const.tile([P, 1], f32)
        nc.vector.memset(eps_sb[:], LN_EPS)

        # raw x^T persists through all QKV projections; o until projection
        xraw_pool = es.enter_context(tc.tile_pool(name="xraw", bufs=1))
        xraw = [xraw_pool.tile([P, NTOK], bf16, tag=f"xraw{dc}",
                               name=f"xraw{dc}")
                for dc in range(DCH)]
        o_pool = es.enter_context(tc.tile_pool(name="o_sb", bufs=1))
        o_sb = [o_pool.tile([P, NTOK], bf16, tag=f"o{p}", name=f"o{p}")
                for p in range(4)]
        # LN stats, replicated across partitions (f32) + transposed columns
        ln_pool = es.enter_context(tc.tile_pool(name="lnkeep", bufs=1))
        mu_sb = ln_pool.tile([P, NTOK], f32, tag="mu")
        rs_sb = ln_pool.tile([P, NTOK], f32, tag="rs")
        muT = ln_pool.tile([P, KT], f32, tag="muT")
        rsT = ln_pool.tile([P, KT], f32, tag="rsT")

        # ---------------- Phase A: LayerNorm stats in x^T layout -----------
        with tc.tile_pool(name="lnps", bufs=1, space="PSUM") as lnps, \
             tc.tile_pool(name="lnsb", bufs=1) as lnsb, \
             tc.tile_pool(name="xsq", bufs=2) as xsq_pool, \
             tc.tile_pool(name="lnd", bufs=1, space="DRAM") as lnd:
            for dc in range(DCH):
                nc.sync.dma_start(xraw[dc][:], xT[dc * P:(dc + 1) * P, :])

            mu_ps = lnps.tile([P, NTOK], f32, tag="mu")
            sq_ps = lnps.tile([P, NTOK], f32, tag="sq")
            # replicated mean: ones(1/D) as stationary, x^T as moving
            for dc in range(DCH):
                for qc in range(NQC):
                    nc.tensor.matmul(
                        mu_ps[:, qc * 512:(qc + 1) * 512],
                        ones_sb[:],
                        xraw[dc][:, qc * 512:(qc + 1) * 512],
                        start=(dc == 0), stop=(dc == DCH - 1),
                    )
            for dc in range(DCH):
                sq = xsq_pool.tile([P, NTOK], bf16, tag="sq")
                nc.scalar.activation(sq[:], xraw[dc][:], AF.Square)
                for qc in range(NQC):
                    nc.tensor.matmul(
                        sq_ps[:, qc * 512:(qc + 1) * 512],
                        ones_sb[:],
                        sq[:, qc * 512:(qc + 1) * 512],
                        start=(dc == 0), stop=(dc == DCH - 1),
                    )

            var_sb = lnsb.tile([P, NTOK], f32, tag="var")
            nc.scalar.copy(mu_sb[:], mu_ps[:])
            nc.vector.tensor_mul(var_sb[:], mu_sb[:], mu_sb[:])
            nc.vector.tensor_sub(var_sb[:], sq_ps[:], var_sb[:])
            # rstd = exp(-0.5 * ln(var + eps)); Ln/Exp share one ACT table set
            nc.scalar.activation(rs_sb[:], var_sb[:], AF.Ln, bias=eps_sb[:, :])
            nc.scalar.activation(rs_sb[:], rs_sb[:], AF.Exp, scale=-0.5)

            # transposed per-token stats columns for the token-major V drain:
            # bounce row 0 through DRAM, read back as [p, kt] = row[kt*128+p]
            std = lnd.tile([1, 2 * NTOK], f32, tag="std")
            nc.sync.dma_start(std[0:1, 0:NTOK], mu_sb[0:1, :])
            nc.sync.dma_start(std[0:1, NTOK:2 * NTOK], rs_sb[0:1, :])
            nc.sync.dma_start(
                muT[:], std[0:1, 0:NTOK]
                .rearrange("o (kt p) -> (o p) kt", p=P))
            nc.sync.dma_start(
                rsT[:], std[0:1, NTOK:2 * NTOK]
                .rearrange("o (kt p) -> (o p) kt", p=P))

        # ---------------- Phase B: V projection (token-major, augmented) ----
        vaug_pool = es.enter_context(tc.tile_pool(name="vaug", bufs=1))
        vaug = vaug_pool.tile([P, KT, 8, HD + 1], bf16, tag="vaug")
        nc.vector.memset(vaug[:, :, :, HD:HD + 1], 1.0)
        with tc.tile_pool(name="wvp", bufs=1) as wv_scope, \
             tc.tile_pool(name="vp_ps", bufs=2, space="PSUM") as vp_ps, \
             tc.tile_pool(name="vtmp", bufs=2) as vtmp_pool:
            wv_sb = wv_scope.tile([P, DCH, INNER_C], bf16, tag="wv")
            nc.sync.dma_start(
                wv_sb[:], wvT.rearrange("(dc p) f -> p dc f", p=P))
            for ktp in range(KT // 2):
                vp = vp_ps.tile([P, 1024], f32, tag="vp")
                for half in range(2):
                    kt = 2 * ktp + half
                    for dc in range(DCH):
                        nc.tensor.matmul(
                            vp[:, half * 512:(half + 1) * 512],
                            xraw[dc][:, kt * P:(kt + 1) * P],
                            wv_sb[:, dc, :],
                            start=(dc == 0), stop=(dc == DCH - 1),
                        )
                for half in range(2):
                    kt = 2 * ktp + half
                    # v = rs_t * (v_raw - mu_t * wsum_f) + cv_f   (rank-1 LN)
                    vt = vtmp_pool.tile([P, INNER_C], f32, tag="vt")
                    nc.vector.scalar_tensor_tensor(
                        vt[:], nwsv_sb[:], muT[:, kt:kt + 1],
                        vp[:, half * 512:(half + 1) * 512], MUL, ADD)
                    nc.vector.scalar_tensor_tensor(
                        vaug[:, kt, :, 0:HD],
                        vt[:].rearrange("p (h f) -> p h f", h=8),
                        rsT[:, kt:kt + 1],
                        cv_sb[:].rearrange("p (h f) -> p h f", h=8),
                        MUL, ADD)

        # ------------- Phase C: QK projection + attention, pipelined -------
        with tc.tile_pool(name="wqk", bufs=2) as wqk_pool, \
             tc.tile_pool(name="qkt", bufs=1) as qk_pool, \
             tc.tile_pool(name="s_ps", bufs=2, space="PSUM") as s_ps_pool, \
             tc.tile_pool(name="oa_ps", bufs=1, space="PSUM") as oa_ps_pool, \
             tc.tile_pool(name="qk_ps", bufs=1, space="PSUM") as qk_ps_pool, \
             tc.tile_pool(name="qtmp", bufs=2) as qtmp_pool, \
             tc.tile_pool(name="p_sb", bufs=3) as p_pool, \
             tc.tile_pool(name="dn", bufs=2) as dn_pool, \
             tc.tile_pool(name="dnd", bufs=2, space="DRAM") as dnd_pool:

            def dma_wqk(h2):
                w = wqk_pool.tile([P, DCH, 512], bf16, tag="wqk",
                                  name=f"wqk{h2}")
                nc.sync.dma_start(
                    w[:, :, 0:256],
                    wqkT[:, h2 * 256:(h2 + 1) * 256]
                    .rearrange("(dc p) f -> p dc f", p=P))
                nc.sync.dma_start(
                    w[:, :, 256:512],
                    wqkT[:, 512 + h2 * 256:512 + (h2 + 1) * 256]
                    .rearrange("(dc p) f -> p dc f", p=P))
                return w

            # QK projection emission, sliced into single-matmul steps plus
            # drain ops, so it interleaves with the prior pair's attention.
            # Per (kind, ch): 16 matmuls into one [P,1024] PSUM tile, then a
            # 3-op rank-1-LN drain:  q = rs * (q_raw - mu * wsum_f) + c_f
            def qk_steps(pair, wqk_sb, qt, kt_sb):
                pl = pair % 2
                for kind, dst in ((0, qt), (1, kt_sb)):
                    fbase = kind * 256 + pl * P
                    col = kind * 4 + pair
                    for ch in range(2):
                        ps = qk_ps_pool.tile([P, 1024], f32, tag="qkp",
                                             name=f"qkp{pair}{kind}{ch}")
                        for dc in range(DCH):
                            for half in range(2):
                                nc.tensor.matmul(
                                    ps[:, half * 512:(half + 1) * 512],
                                    wqk_sb[:, dc, fbase:fbase + P],
                                    xraw[dc][:, ch * 1024 + half * 512:
                                             ch * 1024 + (half + 1) * 512],
                                    start=(dc == 0), stop=(dc == DCH - 1),
                                )
                                yield
                        sl = slice(ch * 1024, (ch + 1) * 1024)
                        qt_t = qtmp_pool.tile([P, 1024], f32, tag="qt")
                        nc.vector.scalar_tensor_tensor(
                            qt_t[:], mu_sb[:, sl], nws_sb[:, col:col + 1],
                            ps[:], MUL, ADD)
                        yield
                        nc.vector.scalar_tensor_tensor(
                            qt_t[:], rs_sb[:, sl], 0.0, qt_t[:], BYP, MUL)
                        yield
                        nc.vector.tensor_scalar_add(
                            dst[:, sl], qt_t[:], cqk_sb[:, col:col + 1])
                        yield

            def attention(pair, qt, kt_sb, next_steps):
                o_t = o_sb[pair]
                for qq in range(NQC):
                    oa = {}
                    for hl in range(2):
                        oa[hl] = oa_ps_pool.tile(
                            [HD + 1, 512], f32, tag=f"oa{hl}", name=f"oa{hl}")
                    pts = {}
                    for ktile in range(KT + 1):
                        # stage S+exp for ktile, PV consumes ktile-1.  Both
                        # heads of the pair share one [P,1024] S tile (each
                        # 512-half sits in its own PSUM bank) so a single
                        # 1024-wide exp serves the pair.
                        if ktile < KT:
                            sp = s_ps_pool.tile([P, 1024], f32, tag="s",
                                                name="s")
                            for hl in range(2):
                                hb = hl * HD
                                nc.tensor.matmul(
                                    sp[:, hl * 512:(hl + 1) * 512],
                                    kt_sb[hb:hb + HD,
                                          ktile * P:(ktile + 1) * P],
                                    qt[hb:hb + HD,
                                       qq * 512:(qq + 1) * 512],
                                    start=True, stop=True,
                                )
                            pt = p_pool.tile([P, 1024], bf16,
                                             tag="p", name="p")
                            nc.scalar.activation(pt[:], sp[:], AF.Exp)
                            pts[ktile] = pt
                        if ktile > 0:
                            for hl in range(2):
                                nc.tensor.matmul(
                                    oa[hl][:],
                                    vaug[:, ktile - 1, 2 * pair + hl, :],
                                    pts[ktile - 1][:, hl * 512:(hl + 1) * 512],
                                    start=(ktile == 1), stop=(ktile == KT),
                                )
                            pts.pop(ktile - 1)
                        if next_steps is not None:
                            next(next_steps, None)
                    # drain O + denominators, then normalize this q-chunk
                    dnq = dn_pool.tile([1, 1024], f32, tag="dnq")
                    for hl in range(2):
                        nc.vector.tensor_copy(
                            o_t[hl * HD:(hl + 1) * HD,
                                qq * 512:(qq + 1) * 512],
                            oa[hl][0:HD, :])
                        nc.vector.tensor_copy(
                            dnq[0:1, hl * 512:(hl + 1) * 512],
                            oa[hl][HD:HD + 1, :])
                    rec = dn_pool.tile([1, 1024], f32, tag="rec")
                    nc.vector.reciprocal(rec[:], dnq[:])
                    dscr = dnd_pool.tile([1, 1024], f32, tag="dscr")
                    nc.sync.dma_start(dscr[:], rec[:])
                    rbc = dn_pool.tile([P, 512], f32, tag="rbc")
                    for hl in range(2):
                        nc.sync.dma_start(
                            rbc[hl * HD:(hl + 1) * HD, :],
                            dscr[0:1, hl * 512:(hl + 1) * 512]
                            .partition_broadcast(HD)
                            .rearrange("p o f -> p (o f)"))
                    nc.vector.tensor_mul(
                        o_t[:, qq * 512:(qq + 1) * 512],
                        o_t[:, qq * 512:(qq + 1) * 512],
                        rbc[:])

            # software pipeline across the 4 pairs
            wqk_sb = {0: dma_wqk(0)}
            qts = {}

            def make_qkt(pair):
                qts[pair] = (
                    qk_pool.tile([P, NTOK], bf16, tag=f"qt{pair % 2}",
                                 name=f"qt{pair}"),
                    qk_pool.tile([P, NTOK], bf16, tag=f"kt{pair % 2}",
                                 name=f"kt{pair}"),
                )

            make_qkt(0)
            for _ in qk_steps(0, wqk_sb[0], *qts[0]):
                pass
            for pair in range(4):
                nxt = None
                if pair < 3:
                    if pair + 1 == 2:
                        wqk_sb[1] = dma_wqk(1)
                    make_qkt(pair + 1)
                    nxt = qk_steps(pair + 1, wqk_sb[(pair + 1) // 2],
                                   *qts[pair + 1])
                attention(pair, *qts[pair], nxt)
                if nxt is not None:
                    for _ in nxt:  # finish any leftover steps
                        pass

        # ---------------- Phase D: output projection ----------------
        with tc.tile_pool(name="wo", bufs=1) as wo_pool, \
             tc.tile_pool(name="proj_ps", bufs=2, space="PSUM") as proj_ps, \
             tc.tile_pool(name="outsb", bufs=2) as out_pool:
            wo_sb = wo_pool.tile([P, 4, D], bf16)
            nc.sync.dma_start(wo_sb[:], woT.rearrange("(pc p) f -> p pc f", p=P))
            for m in range(DCH):
                ps = proj_ps.tile([P, NTOK], f32, tag="proj")
                for pair in range(4):
                    for qc in range(NQC):
                        nc.tensor.matmul(
                            ps[:, qc * 512:(qc + 1) * 512],
                            wo_sb[:, pair, m * P:(m + 1) * P],
                            o_sb[pair][:, qc * 512:(qc + 1) * 512],
                            start=(pair == 0), stop=(pair == 3),
                        )
                ot = out_pool.tile([P, NTOK], f32, tag="out")
                nc.vector.tensor_copy(ot[:], ps[:])
                nc.sync.dma_start(outT[m * P:(m + 1) * P, :], ot[:])


def _prep_inputs(x, ln_gamma, ln_beta, W_qkv, W_out):
    """Build the 8 per-core input maps (host-side, cheap numpy)."""
    import ml_dtypes
    bf = ml_dtypes.bfloat16
    scale = HD ** -0.5
    Wg = (W_qkv * ln_gamma[None, :].astype(np.float32)).astype(np.float32)
    cfull = (W_qkv @ ln_beta.astype(np.float32)).astype(np.float32)  # [3*inner]
    wsum = Wg.sum(1)  # row sums of the folded weights, [3*inner]
    in_maps = []
    for c in range(8):
        bi, hg = c // 2, c % 2
        r0 = hg * INNER_C
        wq = Wg[r0:r0 + INNER_C] * scale
        wk = Wg[1024 + r0:1024 + r0 + INNER_C]
        wv = Wg[2048 + r0:2048 + r0 + INNER_C]
        cq = cfull[r0:r0 + INNER_C] * scale
        ck = cfull[1024 + r0:1024 + r0 + INNER_C]
        cvv = cfull[2048 + r0:2048 + r0 + INNER_C]
        wsq = wsum[r0:r0 + INNER_C] * scale
        wsk = wsum[1024 + r0:1024 + r0 + INNER_C]
        wsv = wsum[2048 + r0:2048 + r0 + INNER_C]
        cqk = np.empty((P, 8), np.float32)
        nwsa = np.empty((P, 8), np.float32)
        for p in range(4):
            cqk[:, p] = cq[p * P:(p + 1) * P]
            cqk[:, 4 + p] = ck[p * P:(p + 1) * P]
            nwsa[:, p] = -wsq[p * P:(p + 1) * P]
            nwsa[:, 4 + p] = -wsk[p * P:(p + 1) * P]
        in_maps.append({
            "onesc": np.full((P, P), 1.0 / D, np.float32).astype(bf),
            "xT": np.ascontiguousarray(x[bi].T).astype(bf),
            "wqkT": np.ascontiguousarray(
                np.concatenate([wq, wk], 0).T).astype(bf),
            "wvT": np.ascontiguousarray(wv.T).astype(bf),
            "woT": np.ascontiguousarray(W_out[:, r0:r0 + INNER_C].T).astype(bf),
            "cqk": cqk,
            "nws": nwsa,
            "cv": cvv.reshape(1, INNER_C),
            "nwsv": (-wsv).reshape(1, INNER_C),
        })
    return in_maps


_NC_CACHE = None


def kernel(x, ln_gamma, ln_beta, W_qkv, W_out):
    from concourse.bass_utils import run_bass_kernel_spmd
    global _NC_CACHE
    x = np.asarray(x, np.float32)
    in_maps = _prep_inputs(
        x, np.asarray(ln_gamma, np.float32), np.asarray(ln_beta, np.float32),
        np.asarray(W_qkv, np.float32), np.asarray(W_out, np.float32))
    if _NC_CACHE is None:
        _NC_CACHE = build_nc()
    res = run_bass_kernel_spmd(_NC_CACHE, in_maps, list(range(8))).results
    b, n, d = x.shape
    out = np.empty((b, n, d), np.float32)
    for bi in range(b):
        out[bi] = (res[2 * bi]["outT"] + res[2 * bi + 1]["outT"]).T
    return out


# revision 12
# speedup vs baseline: 1.0130x; 1.0130x over previous
"""Trainium2 Bass kernel for a dense transformer attention block (v3, bf16).

Reference computation (per batch b, tokens n=2048, d=1024, 16 heads x 64):
    xn  = LayerNorm(x) * gamma + beta
    qkv = xn @ W_qkv^T ;  q,k,v per head
    att = softmax(q k^T / sqrt(hd)) v
    out = concat_heads(att) @ W_out^T

Sharding over 8 cores: data-parallel over the 4 batches x tensor-parallel over
2 head-groups of 8 heads.  Core c handles batch c//2, heads (c%2)*8 ..+8.
Each core produces a partial out^T (its heads' contribution); the host sums
the two partials per batch and transposes back.

v3 design notes:
  - bf16 data path, fp32 PSUM accumulation and LN-stats chain (a bf16 rstd is
    a systematic per-token scale error that q.k doubles and exp amplifies).
  - NO explicit xhat pass: with gamma absorbed into W and the score scale
    folded, and wsum_f = sum_d W[f,d], the QKV projections run on RAW x^T and
    LayerNorm becomes a rank-1 correction applied during the PSUM drain:
        q[f,t] = rs[t]*(q_raw[f,t] - mu[t]*wsum_f) + c_f
    In feature-major layout mu/rs are free-axis tensors (replicated across
    partitions by the ones-matmul) and wsum/c are per-partition scalars, so
    the drain is scalar_tensor_tensor + mult + bias-add.  In token-major
    layout (V) it is two scalar_tensor_tensor ops with mu/rs as per-partition
    columns, obtained by a transposing DRAM bounce of the stats rows.
  - attention per pair: S^T with the two heads of a pair on PE row-groups
    0-63/64-127 (concurrent on HW), exp on ACT fused with the PSUM drain,
    PV with a ones-column on V giving O^T + softmax denominator in one group.
  - O stays SBUF-resident in bf16; final projection reads it directly.
  - the QK projection+drain of pair p+1 is software-pipelined INTO pair p's
    attention k-loop (engines execute their streams in order, so overlap
    must be arranged at emission time).  PSUM: S 2x2, O-accum 2, QK 2 banks.
"""

import numpy as np

import concourse.bass as bass
import concourse.mybir as mybir
import concourse.tile as tile
from concourse.alu_op_type import AluOpType

P = 128
D = 1024            # model dim
NTOK = 2048         # tokens per batch
HD = 64             # head dim
NH = 16             # total heads
NH_CORE = 8         # heads per core
INNER_C = NH_CORE * HD   # 512 inner dims per core
DCH = D // P        # 8 d-chunks of 128
KT = NTOK // P      # 16 token tiles of 128 (attention k)
NQC = NTOK // 512   # 4 q-chunks of 512
LN_EPS = 1e-5

f32 = mybir.dt.float32
bf16 = mybir.dt.bfloat16
AF = mybir.ActivationFunctionType
MUL = AluOpType.mult
ADD = AluOpType.add
BYP = AluOpType.bypass

_WCTR = [0]


def _legalize_waits(nc, max_waits=1):
    """Walrus wait-slot limits are tiny (fp32 matmul: 1). Hoist excess sync
    waits onto preceding same-engine NoOps — engines execute their stream in
    order, so this is semantics-preserving."""
    import bass_rust as _br
    for fn in nc.m.functions:
        for blk in fn.blocks:
            out = []
            for inst in blk.instructions:
                si = getattr(inst, "sync_info", None)
                if si is not None and len(si.on_wait) > max_waits:
                    waits = list(si.on_wait)
                    keep, extra = waits[:max_waits], waits[max_waits:]
                    eng = inst.engine
                    for w in extra:
                        _WCTR[0] += 1
                        nop = mybir.InstNoOp(name=f"WNOP-{_WCTR[0]}",
                                             ins=[], outs=[])
                        nop.engine = eng
                        nop.sync_info = _br.SyncInfo(on_wait=[w], on_update=[])
                        out.append(nop)
                    inst.sync_info = _br.SyncInfo(on_wait=keep,
                                                  on_update=list(si.on_update))
                out.append(inst)
            blk.instructions[:] = out


def build_nc(loop_n=None):
    nc = bass.Bass()

    xT = nc.dram_tensor("xT", [D, NTOK], bf16, kind="ExternalInput")
    # [d, 1024]: cols 0:512 = q feats (8 heads x 64), cols 512:1024 = k feats
    wqkT = nc.dram_tensor("wqkT", [D, 2 * INNER_C], bf16, kind="ExternalInput")
    wvT = nc.dram_tensor("wvT", [D, INNER_C], bf16, kind="ExternalInput")
    woT = nc.dram_tensor("woT", [INNER_C, D], bf16, kind="ExternalInput")
    onesc = nc.dram_tensor("onesc", [P, P], bf16, kind="ExternalInput")
    # per-feature bias (W @ beta): col j<4 -> q pair j, col j>=4 -> k pair j-4
    cqk = nc.dram_tensor("cqk", [P, 8], f32, kind="ExternalInput")
    # negated row-sums of the folded W (rank-1 LN mean correction)
    nws = nc.dram_tensor("nws", [P, 8], f32, kind="ExternalInput")
    cv = nc.dram_tensor("cv", [1, INNER_C], f32, kind="ExternalInput")
    nwsv = nc.dram_tensor("nwsv", [1, INNER_C], f32, kind="ExternalInput")
    outT = nc.dram_tensor("outT", [D, NTOK], f32, kind="ExternalOutput")

    with tile.TileContext(nc) as tc:
        if loop_n:
            with tc.For_i(0, loop_n, 1):
                _emit(nc, tc, xT, wqkT, wvT, woT, cqk, nws, cv, nwsv, onesc, outT)
        else:
            _emit(nc, tc, xT, wqkT, wvT, woT, cqk, nws, cv, nwsv, onesc, outT)
    _legalize_waits(nc)
    return nc


def _emit(nc, tc, xT, wqkT, wvT, woT, cqk, nws, cv, nwsv, onesc, outT):
    from contextlib import ExitStack

    es = ExitStack()
    with es:
        es.enter_context(nc.allow_low_precision(
            reason="bf16 data path; verified against the fp32 reference"))
        const = es.enter_context(tc.tile_pool(name="const", bufs=1))
        ones_sb = const.tile([P, P], bf16)
        nc.sync.dma_start(ones_sb[:], onesc[:])
        cqk_sb = const.tile([P, 8], f32)
        nc.sync.dma_start(cqk_sb[:], cqk[:])
        nws_sb = const.tile([P, 8], f32)
        nc.sync.dma_start(nws_sb[:], nws[:])
        cv_sb = const.tile([P, INNER_C], f32)
        nc.sync.dma_start(
            cv_sb[:],
            cv[0:1, :].partition_broadcast(P).rearrange("p o f -> p (o f)"))
        nwsv_sb = const.tile([P, INNER_C], f32)
        nc.sync.dma_start(
            nwsv_sb[:],
            nwsv[0:1, :].partition_broadcast(P).rearrange("p o f -> p (o f)"))
        eps_sb = const.tile([P, 1], f32)
        nc.vector.memset(eps_sb[:], LN_EPS)

        # raw x^T persists through all QKV projections; o until projection
        xraw_pool = es.enter_context(tc.tile_pool(name="xraw", bufs=1))
        xraw = [xraw_pool.tile([P, NTOK], bf16, tag=f"xraw{dc}",
                               name=f"xraw{dc}")
                for dc in range(DCH)]
        o_pool = es.enter_context(tc.tile_pool(name="o_sb", bufs=1))
        o_sb = [o_pool.tile([P, NTOK], bf16, tag=f"o{p}", name=f"o{p}")
                for p in range(4)]
        # LN stats, replicated across partitions (f32) + transposed columns
        ln_pool = es.enter_context(tc.tile_pool(name="lnkeep", bufs=1))
        mu_sb = ln_pool.tile([P, NTOK], f32, tag="mu")
        rs_sb = ln_pool.tile([P, NTOK], f32, tag="rs")
        muT = ln_pool.tile([P, KT], f32, tag="muT")
        rsT = ln_pool.tile([P, KT], f32, tag="rsT")

        # ---------------- Phase A: LayerNorm stats in x^T layout -----------
        with tc.tile_pool(name="lnps", bufs=1, space="PSUM") as lnps, \
             tc.tile_pool(name="lnsb", bufs=1) as lnsb, \
             tc.tile_pool(name="xsq", bufs=2) as xsq_pool, \
             tc.tile_pool(name="lnd", bufs=1, space="DRAM") as lnd:
            for dc in range(DCH):
                nc.sync.dma_start(xraw[dc][:], xT[dc * P:(dc + 1) * P, :])

            mu_ps = lnps.tile([P, NTOK], f32, tag="mu")
            sq_ps = lnps.tile([P, NTOK], f32, tag="sq")
            # pre-accumulate the 8 d-chunk partials on DVE (bf16, 2x rate) so
            # the replicating ones-matmul contracts once instead of 8 times
            xacc = lnsb.tile([P, NTOK], bf16, tag="xacc")
            sacc = lnsb.tile([P, NTOK], bf16, tag="sacc")
            nc.vector.tensor_add(xacc[:], xraw[0][:], xraw[1][:])
            for dc in range(2, DCH):
                nc.vector.tensor_add(xacc[:], xacc[:], xraw[dc][:])
            sq0 = xsq_pool.tile([P, NTOK], bf16, tag="sq")
            nc.scalar.activation(sq0[:], xraw[0][:], AF.Square)
            sq1 = xsq_pool.tile([P, NTOK], bf16, tag="sq")
            nc.scalar.activation(sq1[:], xraw[1][:], AF.Square)
            nc.vector.tensor_add(sacc[:], sq0[:], sq1[:])
            for dc in range(2, DCH):
                sq = xsq_pool.tile([P, NTOK], bf16, tag="sq")
                nc.scalar.activation(sq[:], xraw[dc][:], AF.Square)
                nc.vector.tensor_add(sacc[:], sacc[:], sq[:])
            # replicated mean / mean-square: ones(1/D) stationary
            for qc in range(NQC):
                nc.tensor.matmul(
                    mu_ps[:, qc * 512:(qc + 1) * 512], ones_sb[:],
                    xacc[:, qc * 512:(qc + 1) * 512], start=True, stop=True)
            for qc in range(NQC):
                nc.tensor.matmul(
                    sq_ps[:, qc * 512:(qc + 1) * 512], ones_sb[:],
                    sacc[:, qc * 512:(qc + 1) * 512], start=True, stop=True)

            var_sb = lnsb.tile([P, NTOK], f32, tag="var")
            nc.scalar.copy(mu_sb[:], mu_ps[:])
            nc.vector.tensor_mul(var_sb[:], mu_sb[:], mu_sb[:])
            nc.vector.tensor_sub(var_sb[:], sq_ps[:], var_sb[:])
            # rstd = exp(-0.5 * ln(var + eps)); Ln/Exp share one ACT table set
            nc.scalar.activation(rs_sb[:], var_sb[:], AF.Ln, bias=eps_sb[:, :])
            nc.scalar.activation(rs_sb[:], rs_sb[:], AF.Exp, scale=-0.5)

            # transposed per-token stats columns for the token-major V drain:
            # bounce row 0 through DRAM, read back as [p, kt] = row[kt*128+p]
            std = lnd.tile([1, 2 * NTOK], f32, tag="std")
            nc.sync.dma_start(std[0:1, 0:NTOK], mu_sb[0:1, :])
            nc.sync.dma_start(std[0:1, NTOK:2 * NTOK], rs_sb[0:1, :])
            nc.sync.dma_start(
                muT[:], std[0:1, 0:NTOK]
                .rearrange("o (kt p) -> (o p) kt", p=P))
            nc.sync.dma_start(
                rsT[:], std[0:1, NTOK:2 * NTOK]
                .rearrange("o (kt p) -> (o p) kt", p=P))

        # ---------------- Phase B: V projection (token-major, augmented) ----
        vaug_pool = es.enter_context(tc.tile_pool(name="vaug", bufs=1))
        vaug = vaug_pool.tile([P, KT, 8, HD + 1], bf16, tag="vaug")
        nc.vector.memset(vaug[:, :, :, HD:HD + 1], 1.0)
        with tc.tile_pool(name="wvp", bufs=1) as wv_scope, \
             tc.tile_pool(name="vp_ps", bufs=2, space="PSUM") as vp_ps, \
             tc.tile_pool(name="vtmp", bufs=2) as vtmp_pool:
            wv_sb = wv_scope.tile([P, DCH, INNER_C], bf16, tag="wv")
            nc.sync.dma_start(
                wv_sb[:], wvT.rearrange("(dc p) f -> p dc f", p=P))
            for ktp in range(KT // 2):
                vp = vp_ps.tile([P, 1024], f32, tag="vp")
                for half in range(2):
                    kt = 2 * ktp + half
                    for dc in range(DCH):
                        nc.tensor.matmul(
                            vp[:, half * 512:(half + 1) * 512],
                            xraw[dc][:, kt * P:(kt + 1) * P],
                            wv_sb[:, dc, :],
                            start=(dc == 0), stop=(dc == DCH - 1),
                        )
                for half in range(2):
                    kt = 2 * ktp + half
                    # v = rs_t * (v_raw - mu_t * wsum_f) + cv_f   (rank-1 LN)
                    vt = vtmp_pool.tile([P, INNER_C], f32, tag="vt")
                    nc.vector.scalar_tensor_tensor(
                        vt[:], nwsv_sb[:], muT[:, kt:kt + 1],
                        vp[:, half * 512:(half + 1) * 512], MUL, ADD)
                    nc.vector.scalar_tensor_tensor(
                        vaug[:, kt, :, 0:HD],
                        vt[:].rearrange("p (h f) -> p h f", h=8),
                        rsT[:, kt:kt + 1],
                        cv_sb[:].rearrange("p (h f) -> p h f", h=8),
                        MUL, ADD)

        # ------------- Phase C: QK projection + attention, pipelined -------
        with tc.tile_pool(name="wqk", bufs=2) as wqk_pool, \
             tc.tile_pool(name="qkt", bufs=1) as qk_pool, \
             tc.tile_pool(name="s_ps", bufs=2, space="PSUM") as s_ps_pool, \
             tc.tile_pool(name="oa_ps", bufs=1, space="PSUM") as oa_ps_pool, \
             tc.tile_pool(name="qk_ps", bufs=1, space="PSUM") as qk_ps_pool, \
             tc.tile_pool(name="qtmp", bufs=2) as qtmp_pool, \
             tc.tile_pool(name="p_sb", bufs=3) as p_pool, \
             tc.tile_pool(name="dn", bufs=2) as dn_pool, \
             tc.tile_pool(name="dnd", bufs=2, space="DRAM") as dnd_pool:

            def dma_wqk(h2):
                w = wqk_pool.tile([P, DCH, 512], bf16, tag="wqk",
                                  name=f"wqk{h2}")
                nc.sync.dma_start(
                    w[:, :, 0:256],
                    wqkT[:, h2 * 256:(h2 + 1) * 256]
                    .rearrange("(dc p) f -> p dc f", p=P))
                nc.sync.dma_start(
                    w[:, :, 256:512],
                    wqkT[:, 512 + h2 * 256:512 + (h2 + 1) * 256]
                    .rearrange("(dc p) f -> p dc f", p=P))
                return w

            # QK projection emission, sliced into single-matmul steps plus
            # drain ops, so it interleaves with the prior pair's attention.
            # Per (kind, ch): 16 matmuls into one [P,1024] PSUM tile, then a
            # 3-op rank-1-LN drain:  q = rs * (q_raw - mu * wsum_f) + c_f
            def qk_steps(pair, wqk_sb, qt, kt_sb):
                pl = pair % 2
                for kind, dst in ((0, qt), (1, kt_sb)):
                    fbase = kind * 256 + pl * P
                    col = kind * 4 + pair
                    for ch in range(2):
                        ps = qk_ps_pool.tile([P, 1024], f32, tag="qkp",
                                             name=f"qkp{pair}{kind}{ch}")
                        for dc in range(DCH):
                            for half in range(2):
                                nc.tensor.matmul(
                                    ps[:, half * 512:(half + 1) * 512],
                                    wqk_sb[:, dc, fbase:fbase + P],
                                    xraw[dc][:, ch * 1024 + half * 512:
                                             ch * 1024 + (half + 1) * 512],
                                    start=(dc == 0), stop=(dc == DCH - 1),
                                )
                                yield
                        sl = slice(ch * 1024, (ch + 1) * 1024)
                        qt_t = qtmp_pool.tile([P, 1024], f32, tag="qt")
                        nc.vector.scalar_tensor_tensor(
                            qt_t[:], mu_sb[:, sl], nws_sb[:, col:col + 1],
                            ps[:], MUL, ADD)
                        yield
                        nc.vector.scalar_tensor_tensor(
                            qt_t[:], rs_sb[:, sl], 0.0, qt_t[:], BYP, MUL)
                        yield
                        nc.vector.tensor_scalar_add(
                            dst[:, sl], qt_t[:], cqk_sb[:, col:col + 1])
                        yield

            def attention(pair, qt, kt_sb, next_steps):
                o_t = o_sb[pair]
                for qq in range(NQC):
                    oa = {}
                    for hl in range(2):
                        oa[hl] = oa_ps_pool.tile(
                            [HD + 1, 512], f32, tag=f"oa{hl}", name=f"oa{hl}")
                    pts = {}
                    for ktile in range(KT + 1):
                        # stage S+exp for ktile, PV consumes ktile-1.  Both
                        # heads of the pair share one [P,1024] S tile (each
                        # 512-half sits in its own PSUM bank) so a single
                        # 1024-wide exp serves the pair.
                        if ktile < KT:
                            sp = s_ps_pool.tile([P, 1024], f32, tag="s",
                                                name="s")
                            for hl in range(2):
                                hb = hl * HD
                                nc.tensor.matmul(
                                    sp[:, hl * 512:(hl + 1) * 512],
                                    kt_sb[hb:hb + HD,
                                          ktile * P:(ktile + 1) * P],
                                    qt[hb:hb + HD,
                                       qq * 512:(qq + 1) * 512],
                                    start=True, stop=True,
                                )
                            pt = p_pool.tile([P, 1024], bf16,
                                             tag="p", name="p")
                            nc.scalar.activation(pt[:], sp[:], AF.Exp)
                            pts[ktile] = pt
                        if ktile > 0:
                            for hl in range(2):
                                nc.tensor.matmul(
                                    oa[hl][:],
                                    vaug[:, ktile - 1, 2 * pair + hl, :],
                                    pts[ktile - 1][:, hl * 512:(hl + 1) * 512],
                                    start=(ktile == 1), stop=(ktile == KT),
                                )
                            pts.pop(ktile - 1)
                        if next_steps is not None:
                            next(next_steps, None)
                    # drain O + denominators, then normalize this q-chunk
                    dnq = dn_pool.tile([1, 1024], f32, tag="dnq")
                    for hl in range(2):
                        nc.vector.tensor_copy(
                            o_t[hl * HD:(hl + 1) * HD,
                                qq * 512:(qq + 1) * 512],
                            oa[hl][0:HD, :])
                        nc.vector.tensor_copy(
                            dnq[0:1, hl * 512:(hl + 1) * 512],
                            oa[hl][HD:HD + 1, :])
                    rec = dn_pool.tile([1, 1024], f32, tag="rec")
                    nc.vector.reciprocal(rec[:], dnq[:])
                    dscr = dnd_pool.tile([1, 1024], f32, tag="dscr")
                    nc.sync.dma_start(dscr[:], rec[:])
                    rbc = dn_pool.tile([P, 512], f32, tag="rbc")
                    for hl in range(2):
                        nc.sync.dma_start(
                            rbc[hl * HD:(hl + 1) * HD, :],
                            dscr[0:1, hl * 512:(hl + 1) * 512]
                            .partition_broadcast(HD)
                            .rearrange("p o f -> p (o f)"))
                    nc.vector.tensor_mul(
                        o_t[:, qq * 512:(qq + 1) * 512],
                        o_t[:, qq * 512:(qq + 1) * 512],
                        rbc[:])

            # software pipeline across the 4 pairs
            wqk_sb = {0: dma_wqk(0)}
            qts = {}

            def make_qkt(pair):
                qts[pair] = (
                    qk_pool.tile([P, NTOK], bf16, tag=f"qt{pair % 2}",
                                 name=f"qt{pair}"),
                    qk_pool.tile([P, NTOK], bf16, tag=f"kt{pair % 2}",
                                 name=f"kt{pair}"),
                )

            make_qkt(0)
            for _ in qk_steps(0, wqk_sb[0], *qts[0]):
                pass
            for pair in range(4):
                nxt = None
                if pair < 3:
                    if pair + 1 == 2:
                        wqk_sb[1] = dma_wqk(1)
                    make_qkt(pair + 1)
                    nxt = qk_steps(pair + 1, wqk_sb[(pair + 1) // 2],
                                   *qts[pair + 1])
                attention(pair, *qts[pair], nxt)
                if nxt is not None:
                    for _ in nxt:  # finish any leftover steps
                        pass

        # ---------------- Phase D: output projection ----------------
        with tc.tile_pool(name="wo", bufs=1) as wo_pool, \
             tc.tile_pool(name="proj_ps", bufs=2, space="PSUM") as proj_ps, \
             tc.tile_pool(name="outsb", bufs=2) as out_pool:
            wo_sb = wo_pool.tile([P, 4, D], bf16)
            nc.sync.dma_start(wo_sb[:], woT.rearrange("(pc p) f -> p pc f", p=P))
            for m in range(DCH):
                ps = proj_ps.tile([P, NTOK], f32, tag="proj")
                for pair in range(4):
                    for qc in range(NQC):
                        nc.tensor.matmul(
                            ps[:, qc * 512:(qc + 1) * 512],
                            wo_sb[:, pair, m * P:(m + 1) * P],
                            o_sb[pair][:, qc * 512:(qc + 1) * 512],
                            start=(pair == 0), stop=(pair == 3),
                        )
                ot = out_pool.tile([P, NTOK], f32, tag="out")
                nc.vector.tensor_copy(ot[:], ps[:])
                nc.sync.dma_start(outT[m * P:(m + 1) * P, :], ot[:])


def _prep_inputs(x, ln_gamma, ln_beta, W_qkv, W_out):
    """Build the 8 per-core input maps (host-side, cheap numpy)."""
    import ml_dtypes
    bf = ml_dtypes.bfloat16
    scale = HD ** -0.5
    Wg = (W_qkv * ln_gamma[None, :].astype(np.float32)).astype(np.float32)
    cfull = (W_qkv @ ln_beta.astype(np.float32)).astype(np.float32)  # [3*inner]
    wsum = Wg.sum(1)  # row sums of the folded weights, [3*inner]
    in_maps = []
    for c in range(8):
        bi, hg = c // 2, c % 2
        r0 = hg * INNER_C
        wq = Wg[r0:r0 + INNER_C] * scale
        wk = Wg[1024 + r0:1024 + r0 + INNER_C]
        wv = Wg[2048 + r0:2048 + r0 + INNER_C]
        cq = cfull[r0:r0 + INNER_C] * scale
        ck = cfull[1024 + r0:1024 + r0 + INNER_C]
        cvv = cfull[2048 + r0:2048 + r0 + INNER_C]
        wsq = wsum[r0:r0 + INNER_C] * scale
        wsk = wsum[1024 + r0:1024 + r0 + INNER_C]
        wsv = wsum[2048 + r0:2048 + r0 + INNER_C]
        cqk = np.empty((P, 8), np.float32)
        nwsa = np.empty((P, 8), np.float32)
        for p in range(4):
            cqk[:, p] = cq[p * P:(p + 1) * P]
            cqk[:, 4 + p] = ck[p * P:(p + 1) * P]
            nwsa[:, p] = -wsq[p * P:(p + 1) * P]
            nwsa[:, 4 + p] = -wsk[p * P:(p + 1) * P]
        in_maps.append({
            "onesc": np.full((P, P), 1.0 / D, np.float32).astype(bf),
            "xT": np.ascontiguousarray(x[bi].T).astype(bf),
            "wqkT": np.ascontiguousarray(
                np.concatenate([wq, wk], 0).T).astype(bf),
            "wvT": np.ascontiguousarray(wv.T).astype(bf),
            "woT": np.ascontiguousarray(W_out[:, r0:r0 + INNER_C].T).astype(bf),
            "cqk": cqk,
            "nws": nwsa,
            "cv": cvv.reshape(1, INNER_C),
            "nwsv": (-wsv).reshape(1, INNER_C),
        })
    return in_maps


_NC_CACHE = None


def kernel(x, ln_gamma, ln_beta, W_qkv, W_out):
    from concourse.bass_utils import run_bass_kernel_spmd
    global _NC_CACHE
    x = np.asarray(x, np.float32)
    in_maps = _prep_inputs(
        x, np.asarray(ln_gamma, np.float32), np.asarray(ln_beta, np.float32),
        np.asarray(W_qkv, np.float32), np.asarray(W_out, np.float32))
    if _NC_CACHE is None:
        _NC_CACHE = build_nc()
    res = run_bass_kernel_spmd(_NC_CACHE, in_maps, list(range(8))).results
    b, n, d = x.shape
    out = np.empty((b, n, d), np.float32)
    for bi in range(b):
        out[bi] = (res[2 * bi]["outT"] + res[2 * bi + 1]["outT"]).T
    return out
